# revision 3
# baseline (speedup 1.0000x reference)
"""Trainium2 Bass kernel for nn_CrossAttention (B=4, Q=1024, T=4096, D=1024, H=16).

Sharding: core = b*2 + g  (b in 0..3 batches, g in 0..1 head-groups of 8 heads).
Each core computes, for its (batch, head-group):
  qT = (Wq_g @ x_q.T)          [512, Q]   (feature-major, heads stacked in pairs)
  kT = (Wk_g @ x_kv.T)         [512, T]
  v  = (x_kv @ Wv_g.T)         [T, 512]
  sT = k_h @ q_h.T             [T, Q] per head  (scores transposed)
  p  = exp(sT / 8)             (softmax without max-subtraction; scores ~N(0,1))
  outT_h = v_h.T @ p ; sums_h = ones.T @ p ; attnT_h = outT_h * (1/sums_h)
  yT_partial = Wo[:, gblock].T.T @ attnT  -> [1024, Q]  fp32
Host sums the two head-group partials per batch and transposes.

Compute dtype bf16 (fp32 PSUM accumulation); softmax denominators in fp32.
"""

import sys

import numpy as np

for _p in ("/opt/trn_rl_repo",):
    if _p not in sys.path:
        sys.path.insert(0, _p)

import ml_dtypes

import concourse.bass as bass
import concourse.tile as tile
from concourse import bacc, mybir
from concourse.bass_utils import run_bass_kernel_spmd

BF16 = mybir.dt.bfloat16
F32 = mybir.dt.float32
NPBF16 = np.dtype(ml_dtypes.bfloat16)

D = 1024          # model dim
Q = 1024          # query length
T = 4096          # kv length
B = 4             # batch
H = 16            # heads
DH = 64           # head dim
NCORES = 8
G = 2             # head groups (cores per batch)
F = D // G        # features per core = 512
P = 128
ND = D // P       # 8 d-tiles (contraction tiles for projections)
NM = F // P       # 4 feature tiles ("pairs" of heads)
NQC = Q // 512    # 2 query chunks
NTC = T // 512    # 8 kv chunks
NTT = T // P      # 32 kv tiles
SCALE = DH ** -0.5


def _emit_kernel(nc, tc, xqT, xkT, wqT, wkT, wvT, woT, yT):
    from contextlib import ExitStack

    ctx = ExitStack()
    with ctx:
        wp = ctx.enter_context(tc.tile_pool(name="wp", bufs=1))
        xp = ctx.enter_context(tc.tile_pool(name="xp", bufs=2))
        st = ctx.enter_context(tc.tile_pool(name="st", bufs=1))
        exp_pool = ctx.enter_context(tc.tile_pool(name="exp", bufs=8))
        small = ctx.enter_context(tc.tile_pool(name="small", bufs=2))
        yop = ctx.enter_context(tc.tile_pool(name="yop", bufs=4))

        # ---- resident weights ----
        wq_sb = wp.tile([P, ND, F], BF16, name="wq_sb", tag="wq")
        wk_sb = wp.tile([P, ND, F], BF16, name="wk_sb", tag="wk")
        wv_sb = wp.tile([P, ND, F], BF16, name="wv_sb", tag="wv")
        wo_sb = wp.tile([P, NM, D], BF16, name="wo_sb", tag="wo")
        for d in range(ND):
            nc.sync.dma_start(out=wq_sb[:, d, :], in_=wqT[d * P:(d + 1) * P, :])
            nc.sync.dma_start(out=wk_sb[:, d, :], in_=wkT[d * P:(d + 1) * P, :])
            nc.sync.dma_start(out=wv_sb[:, d, :], in_=wvT[d * P:(d + 1) * P, :])
        for k in range(NM):
            nc.sync.dma_start(out=wo_sb[:, k, :], in_=woT[k * P:(k + 1) * P, :])

        # ---- resident activations/outputs of projection phase ----
        qT_sb = st.tile([P, NM, Q], BF16, name="qT_sb", tag="qT")
        kT_sb = st.tile([P, NM, T], BF16, name="kT_sb", tag="kT")
        v_sb = st.tile([P, NTT, F], BF16, name="v_sb", tag="v")
        at_sb = st.tile([P, NM, Q], BF16, name="at_sb", tag="at")
        ones_sb = st.tile([P, DH], F32, name="ones_sb", tag="ones")
        ones_col = st.tile([P, 1], BF16, name="ones_col", tag="onec")
        nc.vector.memset(ones_sb, 1.0)
        nc.vector.memset(ones_col, 1.0)

        # ================= projection phase =================
        with tc.tile_pool(name="ppool", bufs=4, space="PSUM") as ppool:
            # q projection: qT[f, q] per q-chunk
            for qc in range(NQC):
                xq_t = xp.tile([P, ND, 512], BF16, name="xq_t", tag="xq")
                for d in range(ND):
                    nc.sync.dma_start(
                        out=xq_t[:, d, :],
                        in_=xqT[d * P:(d + 1) * P, qc * 512:(qc + 1) * 512],
                    )
                for m in range(NM):
                    pq = ppool.tile([P, 512], F32, name="pq", tag="pp")
                    for d in range(ND):
                        nc.tensor.matmul(
                            pq,
                            lhsT=wq_sb[:, d, m * P:(m + 1) * P],
                            rhs=xq_t[:, d, :],
                            start=(d == 0),
                            stop=(d == ND - 1),
                        )
                    nc.vector.tensor_copy(
                        out=qT_sb[:, m, qc * 512:(qc + 1) * 512], in_=pq
                    )
            # k and v projections per kv-chunk
            for tc_i in range(NTC):
                xk_t = xp.tile([P, ND, 512], BF16, name="xk_t", tag="xk")
                for d in range(ND):
                    nc.sync.dma_start(
                        out=xk_t[:, d, :],
                        in_=xkT[d * P:(d + 1) * P, tc_i * 512:(tc_i + 1) * 512],
                    )
                for m in range(NM):
                    pk = ppool.tile([P, 512], F32, name="pk", tag="pp")
                    for d in range(ND):
                        nc.tensor.matmul(
                            pk,
                            lhsT=wk_sb[:, d, m * P:(m + 1) * P],
                            rhs=xk_t[:, d, :],
                            start=(d == 0),
                            stop=(d == ND - 1),
                        )
                    nc.vector.tensor_copy(
                        out=kT_sb[:, m, tc_i * 512:(tc_i + 1) * 512], in_=pk
                    )
                for j in range(4):  # t-tiles within the chunk
                    pv = ppool.tile([P, 512], F32, name="pv", tag="pp")
                    for d in range(ND):
                        nc.tensor.matmul(
                            pv,
                            lhsT=xk_t[:, d, j * P:(j + 1) * P],
                            rhs=wv_sb[:, d, :],
                            start=(d == 0),
                            stop=(d == ND - 1),
                        )
                    nc.vector.tensor_copy(out=v_sb[:, tc_i * 4 + j, :], in_=pv)

        # ================= attention phase =================
        # pairs p = 0..3: features p*128..(p+1)*128 = head A (0:64) + head B (64:128)
        # process pair-groups pg = {0,1} covering pairs (2pg, 2pg+1)
        with tc.tile_pool(name="apsum", bufs=1, space="PSUM") as apool:
            for pg in range(2):
                p0, p1 = 2 * pg, 2 * pg + 1
                for qc in range(NQC):
                    qs = slice(qc * 512, (qc + 1) * 512)
                    pv0 = apool.tile([P, 512], F32, name="pv0", tag="pv0")
                    pv1 = apool.tile([P, 512], F32, name="pv1", tag="pv1")
                    psum = apool.tile([P, 512], F32, name="psum", tag="sum")
                    for t in range(NTT):
                        ts = slice(t * P, (t + 1) * P)
                        ex = []
                        for pi, pp in enumerate((p0, p1)):
                            for hb, (lo, hi) in enumerate(((0, DH), (DH, P))):
                                base = 64 * hb
                                s_ps = apool.tile(
                                    [P, 512], F32, name="s_ps", tag="ps", bufs=4
                                )
                                nc.tensor.matmul(
                                    s_ps,
                                    lhsT=kT_sb[base:base + DH, pp, ts],
                                    rhs=qT_sb[base:base + DH, pp, qs],
                                    start=True,
                                    stop=True,
                                    tile_position=(base, 0),
                                )
                                e = exp_pool.tile(
                                    [P, 512], BF16, name="ex", tag="ex"
                                )
                                nc.scalar.activation(
                                    out=e,
                                    in_=s_ps,
                                    func=mybir.ActivationFunctionType.Exp,
                                    scale=SCALE,
                                )
                                ex.append(e)
                        # PV: col-tiled pairs into pv0/pv1
                        for pi, (pvt, pp) in enumerate(((pv0, p0), (pv1, p1))):
                            nc.tensor.matmul(
                                pvt[0:DH, :],
                                lhsT=v_sb[:, t, pp * P:pp * P + DH],
                                rhs=ex[2 * pi],
                                start=(t == 0),
                                stop=(t == NTT - 1),
                                tile_position=(0, 0),
                            )
                            nc.tensor.matmul(
                                pvt[DH:P, :],
                                lhsT=v_sb[:, t, pp * P + DH:(pp + 1) * P],
                                rhs=ex[2 * pi + 1],
                                start=(t == 0),
                                stop=(t == NTT - 1),
                                tile_position=(0, 64),
                                skip_group_check=True,
                            )
                        # sums: 4 heads col-tiled into one bank (rows 0,32,64,96)
                        for hi in range(4):
                            base = 32 * hi
                            nc.tensor.matmul(
                                psum[base:base + 1, :],
                                lhsT=ones_col,
                                rhs=ex[hi],
                                start=(t == 0),
                                stop=(t == NTT - 1),
                                tile_position=(0, base),
                                skip_group_check=(hi > 0),
                            )
                    # normalize: attnT = outT * (1/sums), per head
                    rc = small.tile([P, 512], F32, name="rc", tag="rc")
                    for hi in range(4):
                        base = 32 * hi
                        nc.vector.reciprocal(
                            out=rc[base:base + 1, :], in_=psum[base:base + 1, :]
                        )
                    for pi, (pvt, pp) in enumerate(((pv0, p0), (pv1, p1))):
                        bc_ps = apool.tile([P, 512], F32, name="bc_ps", tag="bc")
                        ra = 32 * (2 * pi)
                        rb = 32 * (2 * pi + 1)
                        nc.tensor.matmul(
                            bc_ps[0:DH, :],
                            lhsT=ones_sb[ra:ra + 1, :],
                            rhs=rc[ra:ra + 1, :],
                            start=True,
                            stop=True,
                            tile_position=(ra, 0),
                        )
                        nc.tensor.matmul(
                            bc_ps[DH:P, :],
                            lhsT=ones_sb[rb:rb + 1, :],
                            rhs=rc[rb:rb + 1, :],
                            start=True,
                            stop=True,
                            tile_position=(rb, 64),
                        )
                        bc_sb = small.tile([P, 512], F32, name="bc_sb", tag="bcs")
                        nc.vector.tensor_copy(out=bc_sb, in_=bc_ps)
                        nc.vector.tensor_mul(
                            at_sb[:, pp, qs], pvt[:, :], bc_sb
                        )

        # ================= output projection =================
        with tc.tile_pool(name="opsum", bufs=4, space="PSUM") as opool:
            for m8 in range(D // P):
                for qc in range(NQC):
                    py = opool.tile([P, 512], F32, name="py", tag="py")
                    for k in range(NM):
                        nc.tensor.matmul(
                            py,
                            lhsT=wo_sb[:, k, m8 * P:(m8 + 1) * P],
                            rhs=at_sb[:, k, qc * 512:(qc + 1) * 512],
                            start=(k == 0),
                            stop=(k == NM - 1),
                        )
                    y_t = yop.tile([P, 512], F32, name="y_t", tag="y")
                    nc.vector.tensor_copy(out=y_t, in_=py)
                    nc.sync.dma_start(
                        out=yT[m8 * P:(m8 + 1) * P, qc * 512:(qc + 1) * 512],
                        in_=y_t,
                    )


_CACHED_NC = None


def build_program():
    global _CACHED_NC
    if _CACHED_NC is not None:
        return _CACHED_NC
    nc = bacc.Bacc(
        "TRN2", target_bir_lowering=False, debug=False, num_devices=NCORES
    )
    xqT = nc.dram_tensor("xqT", [D, Q], BF16, kind="ExternalInput").ap()
    xkT = nc.dram_tensor("xkT", [D, T], BF16, kind="ExternalInput").ap()
    wqT = nc.dram_tensor("wqT", [D, F], BF16, kind="ExternalInput").ap()
    wkT = nc.dram_tensor("wkT", [D, F], BF16, kind="ExternalInput").ap()
    wvT = nc.dram_tensor("wvT", [D, F], BF16, kind="ExternalInput").ap()
    woT = nc.dram_tensor("woT", [F, D], BF16, kind="ExternalInput").ap()
    yT = nc.dram_tensor("yT", [D, Q], F32, kind="ExternalOutput").ap()
    with tile.TileContext(nc) as tc:
        _emit_kernel(nc, tc, xqT, xkT, wqT, wkT, wvT, woT, yT)
    nc.compile()
    _CACHED_NC = nc
    return nc


def make_in_maps(q_in, kv_in, Wq, Wk, Wv, Wo):
    """Shard + transpose + cast on host. Core = b*2 + g."""
    in_maps = []
    xqTs, xkTs = [], []
    for b in range(B):
        xqTs.append(np.ascontiguousarray(q_in[b].T).astype(NPBF16))
        xkTs.append(np.ascontiguousarray(kv_in[b].T).astype(NPBF16))
    w_parts = []
    for g in range(G):
        blk = slice(g * F, (g + 1) * F)
        w_parts.append(
            dict(
                wqT=np.ascontiguousarray(Wq[blk, :].T).astype(NPBF16),
                wkT=np.ascontiguousarray(Wk[blk, :].T).astype(NPBF16),
                wvT=np.ascontiguousarray(Wv[blk, :].T).astype(NPBF16),
                woT=np.ascontiguousarray(Wo[:, blk].T).astype(NPBF16),
            )
        )
    for b in range(B):
        for g in range(G):
            m = dict(xqT=xqTs[b], xkT=xkTs[b])
            m.update(w_parts[g])
            in_maps.append(m)
    return in_maps


def assemble_output(results):
    """results: list of per-core dicts with 'yT' [D, Q] fp32 partials."""
    out = np.empty((B, Q, D), dtype=np.float32)
    for b in range(B):
        acc = results[2 * b]["yT"] + results[2 * b + 1]["yT"]
        out[b] = acc.T
    return out


def kernel(q_in, kv_in, Wq, Wk, Wv, Wo):
    q_in = np.asarray(q_in, dtype=np.float32)
    kv_in = np.asarray(kv_in, dtype=np.float32)
    Wq = np.asarray(Wq, dtype=np.float32)
    Wk = np.asarray(Wk, dtype=np.float32)
    Wv = np.asarray(Wv, dtype=np.float32)
    Wo = np.asarray(Wo, dtype=np.float32)
    nc = build_program()
    in_maps = make_in_maps(q_in, kv_in, Wq, Wk, Wv, Wo)
    res = run_bass_kernel_spmd(nc, in_maps, list(range(NCORES)))
    return assemble_output(res.results)
